# revision 18
# baseline (speedup 1.0000x reference)
# Sliding-window (tiled tril mask) multi-head attention on 8 TRN2 NeuronCores.
#
# Sharding: head-parallel. Core c owns heads {2c, 2c+1} = rows [128c, 128(c+1))
# of the QKV projection output dim, and the matching 128 columns of Wo.
# Each core computes a full [L, D] partial of the output projection;
# the host sums the 8 partials and adds bo.
#
# Mask structure (from reference): keep score (i, j) iff (i % 512) >= (j % 512).
# In 128-granularity blocks: for key-chunk kc (a = kc % 4), the live query
# columns within each 512-query block are [128a, 512) — computed as one
# matmul rectangle per (kc, q-block), with a triu mask on the first 128 cols.
import sys

sys.path.insert(0, "/opt/trn_rl_repo")

import numpy as np
import concourse.bass as bass
from concourse import mybir, tile
from concourse.bass_utils import run_bass_kernel_spmd
import bass_rust

# `concourse.bass_utils` imports `antenv.axon_hooks` for NTFF tracing under
# axon; this image's antenv lacks that submodule, so register an equivalent
# (ctypes against the injected libaxon_pjrt.so, same ABI as the boot shim).
try:
    import antenv.axon_hooks  # noqa: F401
except ImportError:
    import types as _types
    import contextlib as _ctxlib
    import ctypes as _ctypes
    import os as _os
    import antenv as _antenv

    def _make_ntff_hook():
        so_path = "/opt/axon/libaxon_pjrt.so"
        if not _os.path.exists(so_path):
            return None
        lib = _ctypes.CDLL(so_path)
        if not hasattr(lib, "axon_start_nrt_profile"):
            return None
        lib.axon_start_nrt_profile.argtypes = [
            _ctypes.POINTER(_ctypes.c_int64),
            _ctypes.c_size_t,
        ]
        lib.axon_start_nrt_profile.restype = _ctypes.c_int64
        lib.axon_stop_nrt_profile.argtypes = [_ctypes.c_char_p]
        lib.axon_stop_nrt_profile.restype = _ctypes.c_int64

        @_ctxlib.contextmanager
        def _hook(output_dir, device_ids):
            import jax

            jax.devices()
            if device_ids:
                ids = (_ctypes.c_int64 * len(device_ids))(*device_ids)
                rc = lib.axon_start_nrt_profile(ids, len(device_ids))
            else:
                rc = lib.axon_start_nrt_profile(None, 0)
            if rc != 0:
                raise RuntimeError(f"axon_start_nrt_profile rc={rc}")
            try:
                yield
            finally:
                n = lib.axon_stop_nrt_profile(str(output_dir).encode())
                print(f"profile: {n} file(s) written to {output_dir}")

        return _hook

    _mod = _types.ModuleType("antenv.axon_hooks")
    _NTFF_HOOK = [None]

    def get_axon_ntff_profile_hook():
        if _NTFF_HOOK[0] is None:
            _NTFF_HOOK[0] = _make_ntff_hook()
        return _NTFF_HOOK[0]

    def set_axon_ntff_profile_hook(hook):
        _NTFF_HOOK[0] = hook

    _mod.get_axon_ntff_profile_hook = get_axon_ntff_profile_hook
    _mod.set_axon_ntff_profile_hook = set_axon_ntff_profile_hook
    sys.modules["antenv.axon_hooks"] = _mod
    _antenv.axon_hooks = _mod

F16 = mybir.dt.float16
F32 = mybir.dt.float32
D = 1024
L = 2048
NCORES = 8
AF = mybir.ActivationFunctionType


def _drain_and_barrier_split(self, tick_clock, wait_clock):
    # Walrus in this container rejects a single Drain carrying every sem wait
    # ("Too many sync wait commands"); split the waits over several drains.
    g = tick_clock.global_clock
    n = len(g)
    CHUNK = 1
    for i in range(0, n, CHUNK):
        vec = [g[j] if i <= j < i + CHUNK else 0 for j in range(n)]
        if not any(vec):
            continue
        drain_inst = self.nc.sync.drain(fusable=False)
        wait_clock.add_sem_waits(
            drain_inst.ins, bass_rust.ScopedClock({None: bass_rust.VectorClock(vec)})
        )
    self.nc.all_engine_barrier()
    assert self.sems is not None
    popped = self.nc._tile_sem_poison_stack.pop()
    assert popped is self._sem_poison
    self.nc.clear_and_free_semaphores(list(self.sems.allocated().values()))
    self.nc.all_engine_barrier()


tile.TileContext._drain_and_barrier = _drain_and_barrier_split


NOP_MAX_WAITS = 1


def _split_excess_waits(nc):
    """Walrus in this container has very few sync-command slots per
    instruction (an InstMatmult with 2 waits + 1 update fails codegen).
    Keep at most 1 wait on regular instructions; move the rest onto NoOps
    inserted just before on the same engine (<=4 waits each)."""
    ctr = 0
    f = nc.m.functions[0]
    for bb in f.blocks:
        il = bb.instructions
        i = 0
        while i < len(il):
            inst = il[i]
            tn = type(inst).__name__
            if tn in ("InstISA", "InstEventSemaphore"):
                i += 1
                continue
            si = inst.sync_info
            waits = list(si.on_wait) if si and si.on_wait else []
            if len(waits) > 1:
                keep = waits[-1:]
                excess = waits[:-1]
                pos = i
                for j in range(0, len(excess), NOP_MAX_WAITS):
                    if _os.environ.get("WSPLIT_DRAIN"):
                        nop = mybir.InstDrain(name=f"zz-wsplit-{ctr}")
                    else:
                        nop = mybir.InstNoOp(name=f"zz-wsplit-{ctr}")
                        nop.bass_nofuse = True
                    ctr += 1
                    nop.engine = inst.engine
                    nop.debug = inst.debug
                    nop.sync_info = mybir.SyncInfo(
                        on_wait=excess[j : j + NOP_MAX_WAITS], on_update=[]
                    )
                    il.insert(pos, nop)
                    pos += 1
                    i += 1
                inst.sync_info = mybir.SyncInfo(
                    on_wait=keep, on_update=list(si.on_update or [])
                )
            i += 1


def build_nc(split_waits=True):
    nc = bass.Bass()
    xT_d = nc.declare_dram_parameter("xT", [D, L], F16, isOutput=False)
    wq_d = nc.declare_dram_parameter("wq", [128, D], F16, isOutput=False)
    wk_d = nc.declare_dram_parameter("wk", [128, D], F16, isOutput=False)
    wv_d = nc.declare_dram_parameter("wv", [128, D], F16, isOutput=False)
    wo_d = nc.declare_dram_parameter("wo", [128, D], F16, isOutput=False)
    bq_d = nc.declare_dram_parameter("bq", [1, 128], F16, isOutput=False)
    bk_d = nc.declare_dram_parameter("bk", [1, 128], F16, isOutput=False)
    bv_d = nc.declare_dram_parameter("bv", [1, 128], F16, isOutput=False)
    tri_d = nc.declare_dram_parameter("tri", [128, 128], F16, isOutput=False)
    out_d = nc.declare_dram_parameter("out", [L, D], F16, isOutput=True)

    with tile.TileContext(nc) as tc:
        with (
            tc.tile_pool(name="const", bufs=1) as const,
            tc.tile_pool(name="main", bufs=1) as main,
        ):
            xt = const.tile([128, 8 * L], F16, tag="xt")
            for kc in range(8):
                nc.sync.dma_start(xt[:, kc * L : (kc + 1) * L], xT_d[kc * 128 : (kc + 1) * 128, :])
            wq_sb = const.tile([128, D], F16, tag="wq")
            wk_sb = const.tile([128, D], F16, tag="wk")
            wv_sb = const.tile([128, D], F16, tag="wv")
            wo_sb = const.tile([128, D], F16, tag="wo")
            for sb, d in ((wq_sb, wq_d), (wk_sb, wk_d), (wv_sb, wv_d), (wo_sb, wo_d)):
                nc.sync.dma_start(sb[:, :], d[:, :])
            bq_sb = const.tile([1, 128], F16, tag="bq")
            bk_sb = const.tile([1, 128], F16, tag="bk")
            bv_sb = const.tile([1, 128], F16, tag="bv")
            for sb, d in ((bq_sb, bq_d), (bk_sb, bk_d), (bv_sb, bv_d)):
                nc.sync.dma_start(sb[:, :], d[:, :])
            tri_sb = const.tile([128, 128], F16, tag="tri")
            nc.sync.dma_start(tri_sb[:, :], tri_d[:, :])
            ones_sb = const.tile([1, 512], F16, tag="ones")
            nc.vector.memset(ones_sb[:, :], 1.0)

            q_sb = main.tile([128, L], F16, tag="q")
            k_sb = main.tile([128, L], F16, tag="k")
            # v blocks: [v_h0 (64) | ones (64) | v_h1 (64)] per 128-key chunk
            v_sb = main.tile([128, 16 * 192], F16, tag="v")
            nc.vector.memset(v_sb[:, :], 1.0)
            attnT = main.tile([128, L], F16, tag="attnT")
            recip_a = main.tile([128, 1024], F32, tag="ra")
            recip_b = main.tile([128, 1024], F32, tag="rb")

            # ---------------- QKV projections ----------------
            with tc.tile_pool(name="pps", bufs=4, space="PSUM") as pps:
                for wsb, bsb, dst in ((wq_sb, bq_sb, q_sb), (wk_sb, bk_sb, k_sb)):
                    pss = [
                        pps.tile([128, 512], F32, tag="pk", name=f"ps{n}")
                        for n in range(4)
                    ]
                    for kc in range(8):
                        for n in range(4):
                            nc.tensor.matmul(
                                pss[n][:, :],
                                lhsT=wsb[:, kc * 128 : (kc + 1) * 128],
                                rhs=xt[:, kc * L + n * 512 : kc * L + (n + 1) * 512],
                                start=(kc == 0),
                                stop=False,
                            )
                    for n in range(4):
                        nc.tensor.matmul(
                            pss[n][:, :],
                            lhsT=bsb[0:1, :],
                            rhs=ones_sb[0:1, :],
                            start=False,
                            stop=True,
                        )
                        nc.any.tensor_copy(dst[:, n * 512 : (n + 1) * 512], pss[n][:, :])
                # v in natural [keys, dk] layout
                for lc in range(16):
                    psv = pps.tile([128, 512], F32, tag="pk")
                    for kc in range(8):
                        nc.tensor.matmul(
                            psv[:, 0:128],
                            lhsT=xt[:, kc * L + lc * 128 : kc * L + (lc + 1) * 128],
                            rhs=wv_sb[:, kc * 128 : (kc + 1) * 128],
                            start=(kc == 0),
                            stop=False,
                        )
                    nc.tensor.matmul(
                        psv[:, 0:128],
                        lhsT=ones_sb[0:1, 0:128],
                        rhs=bv_sb[0:1, :],
                        start=False,
                        stop=True,
                    )
                    nc.any.tensor_copy(v_sb[:, lc * 192 : lc * 192 + 64], psv[:, 0:64])
                    nc.any.tensor_copy(v_sb[:, lc * 192 + 128 : lc * 192 + 192], psv[:, 64:128])

            # ---------------- attention (per head, per 1024-query sweep) ----------------
            with (
                tc.tile_pool(name="scp", bufs=2, space="PSUM") as scp,
                tc.tile_pool(name="atp", bufs=2, space="PSUM") as atp,
                tc.tile_pool(name="prp", bufs=3) as prp,
            ):
                for h in range(2):
                    hr = slice(h * 64, (h + 1) * 64)
                    at_r = slice(h * 64, h * 64 + 64)  # rows where attn lands
                    dn_r = slice(64 - 64 * h, 128 - 64 * h)  # rows where denom lands
                    vsel = (0, 128) if h == 0 else (64, 192)
                    for sw in range(2):
                        attn_ps = atp.tile([128, 1024], F32, tag="at")
                        for kc in range(16):
                            a = kc % 4
                            w = 512 - 128 * a
                            sc_ps = scp.tile([128, 1024], F32, tag="sc")
                            for ql in range(2):
                                qg = sw * 2 + ql
                                nc.tensor.matmul(
                                    sc_ps[:, ql * 512 + 128 * a : (ql + 1) * 512],
                                    lhsT=k_sb[hr, kc * 128 : (kc + 1) * 128],
                                    rhs=q_sb[hr, qg * 512 + 128 * a : (qg + 1) * 512],
                                    start=True,
                                    stop=True,
                                )
                            pr = prp.tile([128, 1024], F16, tag="pr")
                            sc_v = sc_ps[:, :].rearrange("p (s t) -> p s t", s=2)[:, :, 128 * a :]
                            pr_v = pr[:, 0 : 2 * w].rearrange("p (s t) -> p s t", s=2)
                            nc.scalar.activation(pr_v, sc_v, AF.Exp, scale=0.125)
                            m_v = pr[:, 0 : 2 * w].rearrange("p (s t) -> p s t", s=2)[:, :, 0:128]
                            tri_b = (
                                tri_sb[:, :]
                                .rearrange("p (o t) -> p o t", o=1)
                                .broadcast_to([128, 2, 128])
                            )
                            nc.vector.tensor_mul(m_v, m_v, tri_b)
                            vb = v_sb[:, kc * 192 + vsel[0] : kc * 192 + vsel[1]]
                            for ql in range(2):
                                nc.tensor.matmul(
                                    attn_ps[:, ql * 512 + 128 * a : (ql + 1) * 512],
                                    lhsT=vb,
                                    rhs=pr[:, ql * w : (ql + 1) * w],
                                    start=(kc == 0),
                                    stop=(kc == 15),
                                )
                        # normalize: denom rows -> reciprocal -> shift to attn rows
                        nc.vector.reciprocal(recip_a[dn_r, :], attn_ps[dn_r, :])
                        nc.sync.dma_start(recip_b[at_r, :], recip_a[dn_r, :])
                        nc.vector.tensor_mul(
                            attnT[at_r, sw * 1024 : (sw + 1) * 1024],
                            attn_ps[at_r, :],
                            recip_b[at_r, :],
                        )

            # ---------------- output projection ----------------
            with (
                tc.tile_pool(name="ops", bufs=2, space="PSUM") as ops,
                tc.tile_pool(name="obp", bufs=3) as obp,
            ):
                for lc in range(16):
                    pso = ops.tile([128, 1024], F32, tag="o")
                    for nh in range(2):
                        nc.tensor.matmul(
                            pso[:, nh * 512 : (nh + 1) * 512],
                            lhsT=attnT[:, lc * 128 : (lc + 1) * 128],
                            rhs=wo_sb[:, nh * 512 : (nh + 1) * 512],
                            start=True,
                            stop=True,
                        )
                    ob = obp.tile([128, 1024], F16, tag="ob")
                    nc.any.tensor_copy(ob[:, :], pso[:, :])
                    nc.sync.dma_start(out_d[lc * 128 : (lc + 1) * 128, :], ob[:, :])
    if split_waits:
        _split_excess_waits(nc)
    return nc


def make_in_maps(x, Wq, bq, Wk, bk, Wv, bv, Wo, bo):
    x0 = np.asarray(x, np.float32)[0]  # [L, D]
    xT = np.ascontiguousarray(x0.T).astype(np.float16)
    tri = np.triu(np.ones((128, 128), np.float16))  # keep iff col >= row

    def wslice(Wm, c):
        # [128, 1024]: row p, col kc*128+e  =  Wm[c*128+e, kc*128+p]
        s = np.asarray(Wm, np.float32)[c * 128 : (c + 1) * 128, :]  # [128e, 1024d]
        a = s.T.reshape(8, 128, 128).transpose(1, 0, 2).reshape(128, 1024)
        return np.ascontiguousarray(a).astype(np.float16)

    def woslice(Wo_, c):
        s = np.asarray(Wo_, np.float32)[:, c * 128 : (c + 1) * 128]  # [1024, 128]
        return np.ascontiguousarray(s.T).astype(np.float16)  # [128 e, 1024 dout]

    in_maps = []
    for c in range(NCORES):
        in_maps.append(
            {
                "xT": xT,
                "wq": wslice(Wq, c),
                "wk": wslice(Wk, c),
                "wv": wslice(Wv, c),
                "wo": woslice(Wo, c),
                "bq": np.asarray(bq, np.float16)[None, c * 128 : (c + 1) * 128],
                "bk": np.asarray(bk, np.float16)[None, c * 128 : (c + 1) * 128],
                "bv": np.asarray(bv, np.float16)[None, c * 128 : (c + 1) * 128],
                "tri": tri,
            }
        )
    return in_maps


_NC_CACHE = None


def kernel(x, Wq, bq, Wk, bk, Wv, bv, Wo, bo, **kw):
    global _NC_CACHE
    if _NC_CACHE is None:
        _NC_CACHE = build_nc()
    nc = _NC_CACHE
    in_maps = make_in_maps(x, Wq, bq, Wk, bk, Wv, bv, Wo, bo)
    res = run_bass_kernel_spmd(nc, in_maps, core_ids=list(range(NCORES)), **kw)
    acc = np.zeros((L, D), np.float32)
    for c in range(NCORES):
        acc += res.results[c]["out"].astype(np.float32)
    acc += np.asarray(bo, np.float32)[None, :]
    return acc[None, :, :].astype(np.float32)


# revision 23
# speedup vs baseline: 1.2087x; 1.2087x over previous
# Sliding-window (tiled tril mask) multi-head attention on 8 TRN2 NeuronCores.
#
# Sharding: head-parallel. Core c owns heads {2c, 2c+1} = rows [128c, 128(c+1))
# of the QKV projection output dim, and the matching 128 columns of Wo.
# Each core computes a full [L, D] partial of the output projection;
# the host sums the 8 partials and adds bo.
#
# Mask structure (from reference): keep score (i, j) iff (i % 512) >= (j % 512).
# In 128-granularity blocks: for key-chunk kc (a = kc % 4), the live query
# columns within each 512-query block are [128a, 512) — computed as one
# matmul rectangle per (kc, q-block), with a triu mask on the first 128 cols.
import sys

sys.path.insert(0, "/opt/trn_rl_repo")

import numpy as np
import concourse.bass as bass
from concourse import mybir, tile
from concourse.bass_utils import run_bass_kernel_spmd
import bass_rust

# `concourse.bass_utils` imports `antenv.axon_hooks` for NTFF tracing under
# axon; this image's antenv lacks that submodule, so register an equivalent
# (ctypes against the injected libaxon_pjrt.so, same ABI as the boot shim).
try:
    import antenv.axon_hooks  # noqa: F401
except ImportError:
    import types as _types
    import contextlib as _ctxlib
    import ctypes as _ctypes
    import os as _os
    import antenv as _antenv

    def _make_ntff_hook():
        so_path = "/opt/axon/libaxon_pjrt.so"
        if not _os.path.exists(so_path):
            return None
        lib = _ctypes.CDLL(so_path)
        if not hasattr(lib, "axon_start_nrt_profile"):
            return None
        lib.axon_start_nrt_profile.argtypes = [
            _ctypes.POINTER(_ctypes.c_int64),
            _ctypes.c_size_t,
        ]
        lib.axon_start_nrt_profile.restype = _ctypes.c_int64
        lib.axon_stop_nrt_profile.argtypes = [_ctypes.c_char_p]
        lib.axon_stop_nrt_profile.restype = _ctypes.c_int64

        @_ctxlib.contextmanager
        def _hook(output_dir, device_ids):
            import jax

            jax.devices()
            if device_ids:
                ids = (_ctypes.c_int64 * len(device_ids))(*device_ids)
                rc = lib.axon_start_nrt_profile(ids, len(device_ids))
            else:
                rc = lib.axon_start_nrt_profile(None, 0)
            if rc != 0:
                raise RuntimeError(f"axon_start_nrt_profile rc={rc}")
            try:
                yield
            finally:
                n = lib.axon_stop_nrt_profile(str(output_dir).encode())
                print(f"profile: {n} file(s) written to {output_dir}")

        return _hook

    _mod = _types.ModuleType("antenv.axon_hooks")
    _NTFF_HOOK = [None]

    def get_axon_ntff_profile_hook():
        if _NTFF_HOOK[0] is None:
            _NTFF_HOOK[0] = _make_ntff_hook()
        return _NTFF_HOOK[0]

    def set_axon_ntff_profile_hook(hook):
        _NTFF_HOOK[0] = hook

    _mod.get_axon_ntff_profile_hook = get_axon_ntff_profile_hook
    _mod.set_axon_ntff_profile_hook = set_axon_ntff_profile_hook
    sys.modules["antenv.axon_hooks"] = _mod
    _antenv.axon_hooks = _mod

F16 = mybir.dt.float16
F32 = mybir.dt.float32
D = 1024
L = 2048
NCORES = 8
AF = mybir.ActivationFunctionType


def _drain_and_barrier_split(self, tick_clock, wait_clock):
    # Walrus in this container rejects a single Drain carrying every sem wait
    # ("Too many sync wait commands"); split the waits over several drains.
    g = tick_clock.global_clock
    n = len(g)
    CHUNK = 1
    for i in range(0, n, CHUNK):
        vec = [g[j] if i <= j < i + CHUNK else 0 for j in range(n)]
        if not any(vec):
            continue
        drain_inst = self.nc.sync.drain(fusable=False)
        wait_clock.add_sem_waits(
            drain_inst.ins, bass_rust.ScopedClock({None: bass_rust.VectorClock(vec)})
        )
    self.nc.all_engine_barrier()
    assert self.sems is not None
    popped = self.nc._tile_sem_poison_stack.pop()
    assert popped is self._sem_poison
    self.nc.clear_and_free_semaphores(list(self.sems.allocated().values()))
    self.nc.all_engine_barrier()


tile.TileContext._drain_and_barrier = _drain_and_barrier_split


NOP_MAX_WAITS = 1


def _split_excess_waits(nc):
    """Walrus in this container has very few sync-command slots per
    instruction (an InstMatmult with 2 waits + 1 update fails codegen).
    Keep at most 1 wait on regular instructions; move the rest onto NoOps
    inserted just before on the same engine (<=4 waits each)."""
    ctr = 0
    f = nc.m.functions[0]
    for bb in f.blocks:
        il = bb.instructions
        i = 0
        while i < len(il):
            inst = il[i]
            tn = type(inst).__name__
            if tn in ("InstISA", "InstEventSemaphore"):
                i += 1
                continue
            si = inst.sync_info
            waits = list(si.on_wait) if si and si.on_wait else []
            if len(waits) > 1:
                keep = waits[-1:]
                excess = waits[:-1]
                pos = i
                for j in range(0, len(excess), NOP_MAX_WAITS):
                    if _os.environ.get("WSPLIT_DRAIN"):
                        nop = mybir.InstDrain(name=f"zz-wsplit-{ctr}")
                    else:
                        nop = mybir.InstNoOp(name=f"zz-wsplit-{ctr}")
                        nop.bass_nofuse = True
                    ctr += 1
                    nop.engine = inst.engine
                    nop.debug = inst.debug
                    nop.sync_info = mybir.SyncInfo(
                        on_wait=excess[j : j + NOP_MAX_WAITS], on_update=[]
                    )
                    il.insert(pos, nop)
                    pos += 1
                    i += 1
                inst.sync_info = mybir.SyncInfo(
                    on_wait=keep, on_update=list(si.on_update or [])
                )
            i += 1


def build_nc(split_waits=True):
    nc = bass.Bass()
    xT_d = nc.declare_dram_parameter("xT", [D, L], F16, isOutput=False)
    wq_d = nc.declare_dram_parameter("wq", [128, D], F16, isOutput=False)
    wk_d = nc.declare_dram_parameter("wk", [128, D], F16, isOutput=False)
    wv_d = nc.declare_dram_parameter("wv", [128, D], F16, isOutput=False)
    wo_d = nc.declare_dram_parameter("wo", [128, D], F16, isOutput=False)
    bq_d = nc.declare_dram_parameter("bq", [1, 128], F16, isOutput=False)
    bk_d = nc.declare_dram_parameter("bk", [1, 128], F16, isOutput=False)
    bv_d = nc.declare_dram_parameter("bv", [1, 128], F16, isOutput=False)
    tri_d = nc.declare_dram_parameter("tri", [128, 128], F16, isOutput=False)
    out_d = nc.declare_dram_parameter("out", [L, D], F16, isOutput=True)

    with tile.TileContext(nc) as tc:
        with (
            tc.tile_pool(name="const", bufs=1) as const,
            tc.tile_pool(name="main", bufs=1) as main,
        ):
            xt = const.tile([128, 8 * L], F16, tag="xt")
            for kc in range(8):
                nc.sync.dma_start(xt[:, kc * L : (kc + 1) * L], xT_d[kc * 128 : (kc + 1) * 128, :])
            wq_sb = const.tile([128, D], F16, tag="wq")
            wk_sb = const.tile([128, D], F16, tag="wk")
            wv_sb = const.tile([128, D], F16, tag="wv")
            wo_sb = const.tile([128, D], F16, tag="wo")
            for sb, d in ((wq_sb, wq_d), (wk_sb, wk_d), (wv_sb, wv_d), (wo_sb, wo_d)):
                nc.sync.dma_start(sb[:, :], d[:, :])
            bq_sb = const.tile([1, 128], F16, tag="bq")
            bk_sb = const.tile([1, 128], F16, tag="bk")
            bv_sb = const.tile([1, 128], F16, tag="bv")
            for sb, d in ((bq_sb, bq_d), (bk_sb, bk_d), (bv_sb, bv_d)):
                nc.sync.dma_start(sb[:, :], d[:, :])
            tri_sb = const.tile([128, 128], F16, tag="tri")
            nc.sync.dma_start(tri_sb[:, :], tri_d[:, :])
            ones_sb = const.tile([1, 512], F16, tag="ones")
            nc.vector.memset(ones_sb[:, :], 1.0)

            q_sb = main.tile([128, L], F16, tag="q")
            k_sb = main.tile([128, L], F16, tag="k")
            vt_sb = main.tile([128, L], F16, tag="vt")
            # v blocks: [v_h0 (64) | ones (64) | v_h1 (64)] per 128-key chunk
            v_sb = main.tile([128, 16 * 192], F16, tag="v")
            nc.vector.memset(v_sb[:, :], 1.0)
            attnT = main.tile([128, L], F16, tag="attnT")
            lnden = main.tile([128, 1024], F32, tag="ra")
            lnden_b = main.tile([128, 1024], F32, tag="rb")
            recip_b = main.tile([128, 1024], F32, tag="rc")

            # ---------------- QKV projections ----------------
            with tc.tile_pool(name="pps", bufs=4, space="PSUM") as pps:
                for wsb, bsb, dst in (
                    (wq_sb, bq_sb, q_sb),
                    (wk_sb, bk_sb, k_sb),
                    (wv_sb, bv_sb, vt_sb),
                ):
                    pss = [
                        pps.tile([128, 512], F32, tag="pk", name=f"ps{n}")
                        for n in range(4)
                    ]
                    for kc in range(8):
                        for n in range(4):
                            nc.tensor.matmul(
                                pss[n][:, :],
                                lhsT=wsb[:, kc * 128 : (kc + 1) * 128],
                                rhs=xt[:, kc * L + n * 512 : kc * L + (n + 1) * 512],
                                start=(kc == 0),
                                stop=False,
                            )
                    for n in range(4):
                        nc.tensor.matmul(
                            pss[n][:, :],
                            lhsT=bsb[0:1, :],
                            rhs=ones_sb[0:1, :],
                            start=False,
                            stop=True,
                        )
                        nc.any.tensor_copy(dst[:, n * 512 : (n + 1) * 512], pss[n][:, :])
                # v natural [keys, dk] via xbar transpose-DMA from vT (fp16)
                for lc in range(16):
                    nc.sync.dma_start(
                        v_sb[:, lc * 192 : lc * 192 + 64],
                        vt_sb[0:64, lc * 128 : (lc + 1) * 128],
                        transpose=True,
                    )
                    nc.sync.dma_start(
                        v_sb[:, lc * 192 + 128 : lc * 192 + 192],
                        vt_sb[64:128, lc * 128 : (lc + 1) * 128],
                        transpose=True,
                    )

            # ---------------- attention (per head, per 1024-query sweep) ----------------
            with (
                tc.tile_pool(name="scp", bufs=1, space="PSUM") as scp,
                tc.tile_pool(name="atp", bufs=1, space="PSUM") as atp,
                tc.tile_pool(name="prp", bufs=3) as prp,
            ):
                tri_b = (
                    tri_sb[:, :]
                    .rearrange("p (o t) -> p o t", o=1)
                    .broadcast_to([128, 2, 128])
                )
                for sw in range(2):
                    at_ps = [
                        atp.tile([128, 1024], F32, tag=f"at{h}", name=f"at{h}")
                        for h in range(2)
                    ]
                    for kc in range(16):
                        a = kc % 4
                        w = 512 - 128 * a
                        prs = []
                        for h in range(2):
                            hr = slice(h * 64, (h + 1) * 64)
                            sc_ps = scp.tile(
                                [128, 1024], F32, tag=f"sc{h}", name=f"sc{h}"
                            )
                            for ql in range(2):
                                qg = sw * 2 + ql
                                nc.tensor.matmul(
                                    sc_ps[:, ql * 512 + 128 * a : (ql + 1) * 512],
                                    lhsT=k_sb[hr, kc * 128 : (kc + 1) * 128],
                                    rhs=q_sb[hr, qg * 512 + 128 * a : (qg + 1) * 512],
                                    start=True,
                                    stop=True,
                                )
                            pr = prp.tile([128, 1024], F16, tag=f"pr{h}", name=f"pr{h}")
                            sc_v = sc_ps[:, :].rearrange("p (s t) -> p s t", s=2)[
                                :, :, 128 * a :
                            ]
                            pr_v = pr[:, 0 : 2 * w].rearrange("p (s t) -> p s t", s=2)
                            nc.scalar.activation(pr_v, sc_v, AF.Exp, scale=0.125)
                            m_v = pr[:, 0 : 2 * w].rearrange("p (s t) -> p s t", s=2)[
                                :, :, 0:128
                            ]
                            nc.vector.tensor_mul(m_v, m_v, tri_b)
                            prs.append(pr)
                        for h in range(2):
                            vsel = (0, 128) if h == 0 else (64, 192)
                            vb = v_sb[:, kc * 192 + vsel[0] : kc * 192 + vsel[1]]
                            for ql in range(2):
                                nc.tensor.matmul(
                                    at_ps[h][:, ql * 512 + 128 * a : (ql + 1) * 512],
                                    lhsT=vb,
                                    rhs=prs[h][:, ql * w : (ql + 1) * w],
                                    start=(kc == 0),
                                    stop=(kc == 15),
                                )
                    # normalize: denom rows -> 1/x = exp(-ln(x)) on ACT -> attn rows
                    for h in range(2):
                        at_r = slice(h * 64, h * 64 + 64)
                        dn_r = slice(64 - 64 * h, 128 - 64 * h)
                        nc.scalar.activation(
                            lnden[dn_r, :], at_ps[h][dn_r, :], AF.Ln
                        )
                        nc.sync.dma_start(lnden_b[at_r, :], lnden[dn_r, :])
                        nc.scalar.activation(
                            recip_b[at_r, :], lnden_b[at_r, :], AF.Exp, scale=-1.0
                        )
                        nc.vector.tensor_mul(
                            attnT[at_r, sw * 1024 : (sw + 1) * 1024],
                            at_ps[h][at_r, :],
                            recip_b[at_r, :],
                        )

            # ---------------- output projection ----------------
            with (
                tc.tile_pool(name="ops", bufs=2, space="PSUM") as ops,
                tc.tile_pool(name="obp", bufs=3) as obp,
            ):
                for lc in range(16):
                    pso = ops.tile([128, 1024], F32, tag="o")
                    for nh in range(2):
                        nc.tensor.matmul(
                            pso[:, nh * 512 : (nh + 1) * 512],
                            lhsT=attnT[:, lc * 128 : (lc + 1) * 128],
                            rhs=wo_sb[:, nh * 512 : (nh + 1) * 512],
                            start=True,
                            stop=True,
                        )
                    ob = obp.tile([128, 1024], F16, tag="ob")
                    nc.any.tensor_copy(ob[:, :], pso[:, :])
                    nc.sync.dma_start(out_d[lc * 128 : (lc + 1) * 128, :], ob[:, :])
    if split_waits:
        _split_excess_waits(nc)
    return nc


def make_in_maps(x, Wq, bq, Wk, bk, Wv, bv, Wo, bo):
    x0 = np.asarray(x, np.float32)[0]  # [L, D]
    xT = np.ascontiguousarray(x0.T).astype(np.float16)
    tri = np.triu(np.ones((128, 128), np.float16))  # keep iff col >= row

    def wslice(Wm, c):
        # [128, 1024]: row p, col kc*128+e  =  Wm[c*128+e, kc*128+p]
        s = np.asarray(Wm, np.float32)[c * 128 : (c + 1) * 128, :]  # [128e, 1024d]
        a = s.T.reshape(8, 128, 128).transpose(1, 0, 2).reshape(128, 1024)
        return np.ascontiguousarray(a).astype(np.float16)

    def woslice(Wo_, c):
        s = np.asarray(Wo_, np.float32)[:, c * 128 : (c + 1) * 128]  # [1024, 128]
        return np.ascontiguousarray(s.T).astype(np.float16)  # [128 e, 1024 dout]

    in_maps = []
    for c in range(NCORES):
        in_maps.append(
            {
                "xT": xT,
                "wq": wslice(Wq, c),
                "wk": wslice(Wk, c),
                "wv": wslice(Wv, c),
                "wo": woslice(Wo, c),
                "bq": np.asarray(bq, np.float16)[None, c * 128 : (c + 1) * 128],
                "bk": np.asarray(bk, np.float16)[None, c * 128 : (c + 1) * 128],
                "bv": np.asarray(bv, np.float16)[None, c * 128 : (c + 1) * 128],
                "tri": tri,
            }
        )
    return in_maps


_NC_CACHE = None


def kernel(x, Wq, bq, Wk, bk, Wv, bv, Wo, bo, **kw):
    global _NC_CACHE
    if _NC_CACHE is None:
        _NC_CACHE = build_nc()
    nc = _NC_CACHE
    in_maps = make_in_maps(x, Wq, bq, Wk, bk, Wv, bv, Wo, bo)
    res = run_bass_kernel_spmd(nc, in_maps, core_ids=list(range(NCORES)), **kw)
    acc = np.zeros((L, D), np.float32)
    for c in range(NCORES):
        acc += res.results[c]["out"].astype(np.float32)
    acc += np.asarray(bo, np.float32)[None, :]
    return acc[None, :, :].astype(np.float32)
